# revision 1
# baseline (speedup 1.0000x reference)
"""DPS perturbed-top-k patch-extraction kernel for Trainium2 (Bass/Tile).

Contract: kernel(**inputs) takes the FULL inputs
    x_high  (8, 3, 512, 512) f32
    scores_2d (8, 16, 16) f32
    noise   (8, 500, 256) f32
and returns the FULL output (128, 3, 64, 64) f32.

Sharding: pure data-parallel over batch b across the 8 NeuronCores (one
image per core). No collectives.

Per-core algorithm (matches the reference bit-closely):
  1. min-max normalize scores  s = (sc - min) * recip(max - min + 1e-5)
  2. pert[n,d] = s[d] + 0.05*noise[n,d]     (500, 256)
  3. top-16 threshold per row via vector max8 -> match_replace -> max8
     (exact: verified no fp32 ties at the rank-16/17 boundary for this input)
  4. A = pert >= t written into an 18-stride embedded axis d' = 18*i + j
     (324 wide); cnt = cumsum(A) along d' via tensor_tensor_scan
  5. S'_k[d'] = sum_n f_k(cnt) via transpose + per-k accumulate;
     G_k = S'_k - S'_{k+1};  ind_k[d'] = (G_k[d'] - G_k[d'-1]) / 500
     (any per-k d'-constant offset cancels in the d'-difference, which lets
     ACT use relu(cnt-k) and DVE use max(cnt,k) interchangeably)
  6. out[k] = sum_{i,j} ind[k,18i+j] * patch(i,j) computed as a single
     18x18-block matmul: out_q[(q,k), (c,h',w')] = IND^T @ B with
     B[(a,b), (c,h',w')] = x_pad 32x32 blocks (no overlap redundancy) and
     IND the indicator tile shifted by (18*hq + wq) partitions per output
     quadrant q. f32r matmul (1 cyc/row) with optional fp32 fallback.
"""
import os
import numpy as np
from contextlib import ExitStack

# ---- problem constants (hardcoded per spec) ----
NB = 8           # batch / cores
C = 3
H = W = 512
GS = 16          # score grid 16x16
D2 = 256         # compact d
GE = 18          # embedded grid stride
D3 = GE * GE     # 324
K = 16
N = 500
NCH = 4          # n chunks
NP = 125         # rows per chunk
CM = 108         # partitions per block-chunk (6*18)
PATCH = 64
BLK = 32         # block size (stride between patches)
SIG = 0.05
INV_N = 1.0 / 500.0
NEG = -1.0e30
FREE_B = C * BLK * BLK   # 3072 floats per block partition
NSL = 6                  # 512-wide free slices of FREE_B
O_ROW = C * PATCH * PATCH  # 12288 floats per output patch

_CACHE = {}


def _build_nc():
    import concourse.bacc as bacc
    import concourse.bass as bass
    import concourse.mybir as mybir
    import concourse.tile as tile

    F32 = mybir.dt.float32
    F32R = mybir.dt.float32r
    BF16 = mybir.dt.bfloat16
    I32 = mybir.dt.int32
    ALU = mybir.AluOpType
    ACTF = mybir.ActivationFunctionType
    AP = bass.AP

    use_f32r = os.environ.get("DPS_FP32_MM", "0") != "1"
    MMT = F32R if use_f32r else F32

    nc = bacc.Bacc("TRN2", target_bir_lowering=False, debug=False)
    x_d = nc.dram_tensor("x", (C, H, W), F32, kind="ExternalInput")
    sc_d = nc.dram_tensor("sc", (GS, GS), F32, kind="ExternalInput")
    nz_d = nc.dram_tensor("nz", (N, D2), F32, kind="ExternalInput")
    o_d = nc.dram_tensor("o", (K, O_ROW), F32, kind="ExternalOutput")

    with tile.TileContext(nc) as tc, ExitStack() as ctx:
        sb = ctx.enter_context(tc.tile_pool(name="sb", bufs=1))
        ps_rep = ctx.enter_context(tc.tile_pool(name="ps_rep", bufs=1, space="PSUM"))
        ps_cnt = ctx.enter_context(tc.tile_pool(name="ps_cnt", bufs=1, space="PSUM"))
        ps_out = ctx.enter_context(tc.tile_pool(name="ps_out", bufs=3, space="PSUM"))

        def ap_of(t, off_elems, dims):
            return AP(t.tensor, t[:].offset + off_elems, dims)

        # Round-robin dma_start across the two HWDGE queues (SP + ACT):
        # per-queue dispatch is the dominant serial cost for this kernel.
        def dma(dst, src):
            return nc.sync.dma_start(dst, src)

        def dma_act(dst, src):
            return nc.scalar.dma_start(dst, src)

        def dma_gp(dst, src):
            return nc.gpsimd.dma_start(dst, src)

        # ---------------- B tiles: 18x18 grid of 32x32 blocks ----------------
        # Free layout (h', c, w') so each block row is a contiguous 384B run
        # of the DRAM staging tensor x_pad2[R, b, c, w'] (row-padded, column-
        # block-swizzled). Staging costs ~10 DMAs; B then loads in 18 clean
        # DMAs with no edge cases and no SBUF memsets. dma_start dispatch is
        # globally serialized (~0.65us each) so DMA COUNT dominates the wall.
        B = [sb.tile([CM, FREE_B], F32, tag=f"B{m}", name=f"B{m}") for m in range(3)]
        XPROW = GE * 96            # 1728 floats per padded row
        xp2 = nc.dram_tensor("xp2", (576 * XPROW,), F32, kind="Internal")
        zrow = sb.tile([CM, 2304], F32)
        nc.vector.memset(zrow[:], 0.0)
        # zero-fill all of x_pad2 (995328 elems = 4 * 108 * 2304)
        dma(AP(xp2, 0, [[2304, CM], [248832, 4], [1, 2304]]),
            AP(zrow.tensor, zrow[:].offset, [[2304, CM], [0, 4], [1, 2304]]))
        # interior copy x -> x_pad2: R = r+16; C = 32b + w' - 16
        for c in range(C):
            # main: C_img 16..495  -> b 1..15, w' 0..31 (contiguous 480 runs)
            dma_gp(AP(xp2, 16 * XPROW + 96 + c * 32, [[XPROW, H], [96, 15], [1, 32]]),
                   AP(x_d, c * H * W + 16, [[W, H], [32, 15], [1, 32]]))
        # left edge (all c): C_img 0..15 -> b 0, w' 16..31
        dma(AP(xp2, 16 * XPROW + 16, [[32, C], [XPROW, H], [1, 16]]),
            AP(x_d, 0, [[H * W, C], [W, H], [1, 16]]))
        # right edge (all c): C_img 496..511 -> b 16, w' 0..15
        dma(AP(xp2, 16 * XPROW + 16 * 96, [[32, C], [XPROW, H], [1, 16]]),
            AP(x_d, 496, [[H * W, C], [W, H], [1, 16]]))
        # B loads: one DMA per block row a
        for a in range(18):
            m, a2 = divmod(a, 6)
            dma(ap_of(B[m], (GE * a2) * FREE_B, [[FREE_B, GE], [96, BLK], [1, 96]]),
                AP(xp2, (BLK * a) * XPROW, [[96, GE], [XPROW, BLK], [1, 96]]))

        # ---------------- scores normalization ----------------
        s256 = sb.tile([1, D2], F32)
        dma_act(s256[:], sc_d[:].rearrange("a b -> (a b)").unsqueeze(0))
        smax = sb.tile([1, 1], F32)
        smin = sb.tile([1, 1], F32)
        nc.vector.tensor_reduce(smax[:], s256[:], axis=mybir.AxisListType.X,
                                op=ALU.max)
        nc.vector.tensor_reduce(smin[:], s256[:], axis=mybir.AxisListType.X,
                                op=ALU.min)
        Dt = sb.tile([1, 1], F32)
        nc.vector.tensor_scalar(Dt[:], smax[:], smin[:], 1e-5,
                                op0=ALU.subtract, op1=ALU.add)
        rD = sb.tile([1, 1], F32)
        nc.vector.reciprocal(rD[:], Dt[:])
        s_row = sb.tile([1, D2], F32)
        nc.vector.tensor_scalar(s_row[:], s256[:], smin[:], rD[:],
                                op0=ALU.subtract, op1=ALU.mult)

        ones = sb.tile([1, 128], F32)
        nc.vector.memset(ones[:], 1.0)

        # ---------------- identity + bias tables ----------------
        iota_t = sb.tile([128, 128], I32)
        nc.gpsimd.iota(iota_t[:], pattern=[[-1, 128]], base=0,
                       channel_multiplier=1)
        ident = sb.tile([128, 128], BF16)
        nc.vector.tensor_scalar(ident[:], iota_t[:], 0, None, op0=ALU.is_equal)
        # 0.05 * identity (fp32) for the pert-by-matmul trick
        diag05 = sb.tile([128, 128], F32)
        nc.vector.tensor_scalar(diag05[:], iota_t[:], 0, SIG,
                                op0=ALU.is_equal, op1=ALU.mult)
        ident_f32 = sb.tile([128, 128], F32)
        nc.vector.tensor_scalar(ident_f32[:], iota_t[:], 0, None,
                                op0=ALU.is_equal)
        bias_i = sb.tile([128, 17], I32)
        nc.gpsimd.iota(bias_i[:], pattern=[[-1, 17]], base=0,
                       channel_multiplier=0)
        bias_f = sb.tile([128, 17], F32)
        nc.vector.tensor_copy(bias_f[:], bias_i[:])

        # ---------------- per-chunk top-k threshold + cnt ----------------
        cnt = [sb.tile([128, D3], BF16, tag=f"cnt{t}", name=f"cnt{t}") for t in range(NCH)]
        cntT = [ps_cnt.tile([CM, 512], BF16, tag=f"cntT{m}", name=f"cntT{m}") for m in range(3)]
        for t in range(NCH):
            nz_t = sb.tile([128, D2], F32, tag=f"nz{t}", name=f"nzt{t}")
            dma_act(nz_t[0:NP, :], nz_d[NP * t:NP * (t + 1), :])
            pert_ps = ps_rep.tile([128, D2], F32, tag="pert_ps",
                                  name=f"pert_ps{t}", bufs=2)
            nc.tensor.matmul(pert_ps[0:NP, :], ones[:, 0:NP], s_row[:],
                             start=True, stop=False)
            nc.tensor.matmul(pert_ps[0:NP, :], diag05[0:NP, 0:NP],
                             nz_t[0:NP, :], start=False, stop=True)
            pert = sb.tile([128, D2], F32, tag=f"pert{t}", name=f"pert{t}")
            if t % 2 == 0:
                nc.scalar.copy(pert[0:NP, :], pert_ps[0:NP, :])
            else:
                nc.vector.tensor_copy(pert[0:NP, :], pert_ps[0:NP, :])
            top8 = sb.tile([128, 8], F32, tag=f"top8{t}", name=f"top8_{t}")
            nc.vector.max(top8[0:NP, :], pert[0:NP, :])
            pert2 = sb.tile([128, D2], F32, tag=f"pert2{t}", name=f"pert2_{t}")
            nc.vector.match_replace(pert2[0:NP, :], top8[0:NP, :],
                                    pert[0:NP, :], NEG)
            top8b = sb.tile([128, 8], F32, tag=f"top8b{t}", name=f"top8b_{t}")
            nc.vector.max(top8b[0:NP, :], pert2[0:NP, :])

            A = sb.tile([128, D3], F32, tag=f"A{t}", name=f"A{t}")
            nc.gpsimd.memset(A[:], 0.0)
            # strided write of the compact 256 into the 18-stride embedding
            a_view = ap_of(A, 0, [[D3, NP], [GE, GS], [1, GS]])
            p_view = ap_of(pert, 0, [[D2, NP], [GS, GS], [1, GS]])
            nc.vector.tensor_scalar(a_view, p_view, top8b[0:NP, 7:8], None,
                                    op0=ALU.is_ge)
            nc.vector.memset(cnt[t][:], 0.0)
            nc.vector.tensor_tensor_scan(cnt[t][0:NP, :], A[0:NP, :],
                                         A[0:NP, :], initial=0.0,
                                         op0=ALU.add, op1=ALU.bypass)
        # transposes into PSUM (d' on partitions)
        for t in range(NCH):
            for m in range(3):
                nc.tensor.transpose(
                    cntT[m][:, 128 * t:128 * (t + 1)],
                    cnt[t][:, CM * m:CM * (m + 1)], ident[:])

        # ---------------- S' accumulations ----------------
        Sp = [sb.tile([CM, 17], F32, tag=f"Sp{m}", name=f"Sp{m}") for m in range(3)]
        scr_a = sb.tile([CM, 512], BF16, tag="scr_a", name="scr_a")
        scr_v = sb.tile([CM, 512], BF16, tag="scr_v", name="scr_v")
        for m in range(3):
            nc.vector.memset(Sp[m][:, 16:17], 0.0)
            for k in range(16):
                if k % 2 == 0:
                    nc.scalar.activation(
                        scr_a[:], cntT[m][:], ACTF.Relu,
                        bias=bias_f[0:CM, k:k + 1], scale=1.0,
                        accum_out=Sp[m][:, k:k + 1])
                else:
                    nc.vector.tensor_scalar(
                        scr_v[:], cntT[m][:], float(k), None,
                        op0=ALU.max, op1=ALU.add,
                        accum_out=Sp[m][:, k:k + 1])

        # ---------------- G -> indicators (transpose-based, no DMAs) -----
        # Gc[m][p, k] = (S'_k - S'_{k+1})(d'=108m+p) / 500.  Transposing to
        # k-partitions makes both the d'-1 difference and the per-quadrant
        # d'-shifts FREE-axis offsets, so the whole indicator assembly needs
        # zero DMA dispatches (the SP sequencer is this kernel's bottleneck).
        Gc = [sb.tile([CM, K], F32, tag=f"Gc{m}", name=f"Gc{m}") for m in range(3)]
        for m in range(3):
            g = sb.tile([CM, K], F32, tag=f"G{m}", name=f"G{m}")
            nc.vector.tensor_tensor(g[:], Sp[m][:, 0:16], Sp[m][:, 1:17],
                                    op=ALU.subtract)
            nc.vector.tensor_scalar_mul(Gc[m][:], g[:], INV_N)
        # GcT: (16 k-partitions, 1 + 324) with col 0 = "Gc[-1]" carrying the
        # per-k d'-constant offsets of the mixed relu/max accumulation forms
        gct_sb = sb.tile([16, 1 + D3], F32)
        # "Gc[-1]" column: per-k d'-constant offsets of the mixed relu/max
        # accumulation forms (ACT relu-form even k: 0; DVE max-form odd k:
        # 512k), scaled by 1/500.
        ik_i = sb.tile([16, 1], I32)
        nc.gpsimd.iota(ik_i[:], pattern=[[1, 1]], base=0, channel_multiplier=1)
        ikf = sb.tile([16, 1], F32)
        nc.vector.tensor_copy(ikf[:], ik_i[:])
        par_i = sb.tile([16, 1], I32)
        nc.vector.tensor_scalar(par_i[:], ik_i[:], 1, None, op0=ALU.bitwise_and)
        parf = sb.tile([16, 1], F32)
        nc.vector.tensor_copy(parf[:], par_i[:])
        t1 = sb.tile([16, 1], F32)
        nc.vector.tensor_tensor(t1[:], ikf[:], parf[:], op=ALU.mult)
        u = sb.tile([16, 1], F32)
        nc.vector.tensor_scalar(u[:], parf[:], -1.0, 1.0, op0=ALU.mult,
                                op1=ALU.add)
        ik1 = sb.tile([16, 1], F32)
        nc.vector.tensor_scalar_add(ik1[:], ikf[:], 1.0)
        t2 = sb.tile([16, 1], F32)
        nc.vector.tensor_tensor(t2[:], ik1[:], u[:], op=ALU.mult)
        t3 = sb.tile([16, 1], F32)
        nc.vector.tensor_tensor(t3[:], t1[:], t2[:], op=ALU.subtract)
        nc.vector.tensor_scalar(gct_sb[:, 0:1], t3[:], 512.0 * INV_N, None,
                                op0=ALU.mult)
        for m in range(3):
            gct_ps = ps_rep.tile([16, CM], F32, tag="pert_ps",
                                 name=f"gct{m}", bufs=2)
            nc.tensor.transpose(gct_ps[:], Gc[m][:], ident_f32[0:CM, 0:CM])
            nc.scalar.copy(gct_sb[:, 1 + CM * m:1 + CM * (m + 1)], gct_ps[:])
        # indT_pad: cols 0..18 zero (for quadrant shifts), col 19+d' = ind(d')
        indT_pad = sb.tile([16, 19 + D3], F32)
        nc.vector.memset(indT_pad[:, 0:19], 0.0)
        nc.vector.tensor_tensor(indT_pad[:, 19:19 + D3], gct_sb[:, 1:1 + D3],
                                gct_sb[:, 0:D3], op=ALU.subtract)
        # back-transposes: IND_ps[m][p, 16q+k] = ind(108m + p - (18hq+wq))
        MMT2 = F32R if use_f32r else F32
        INDr = [sb.tile([CM, 64], MMT2, tag=f"INDr{m}", name=f"INDr{m}")
                for m in range(3)]
        for m in range(3):
            ind_ps = ps_cnt.tile([CM, 64], F32, tag=f"cntT{m}",
                                 name=f"indps{m}")
            for hq in range(2):
                for wq in range(2):
                    q = 2 * hq + wq
                    d = GE * hq + wq
                    nc.tensor.transpose(
                        ind_ps[:, 16 * q:16 * (q + 1)],
                        indT_pad[:, 19 + CM * m - d:19 + CM * (m + 1) - d],
                        ident_f32[0:16, 0:16])
            nc.vector.tensor_copy(INDr[m][:], ind_ps[:])

        # ---------------- main matmul + output ----------------
        # rounding copies to f32r for the 1-cyc/row PE path (walrus requires
        # f32r matmul operands to be produced as f32r)
        if use_f32r:
            Br = [sb.tile([CM, FREE_B], F32R, tag=f"Br{m}", name=f"Br{m}")
                  for m in range(3)]
            nc.vector.tensor_copy(Br[0][:], B[0][:])
            nc.vector.tensor_copy(Br[1][:], B[1][:])
            nc.scalar.copy(Br[2][:], B[2][:])
        else:
            Br = B
        # B free layout is (h', c, w'); slice at 480 = 5 h'-rows per matmul
        # so each PSUM tile maps to whole h'-rows. The PSUM->SBUF copy
        # permutes into the output's (c, h, w) order.
        osb = sb.tile([64, O_ROW // 4], F32)   # (qk, c*64*... ) = (64, 3072)
        for t in range(7):
            ncol = 480 if t < 6 else 192
            nrow = ncol // 96
            mm = ps_out.tile([64, 480], F32, tag="mm", name=f"mm{t}")
            for m in range(3):
                nc.tensor.matmul(mm[:, 0:ncol], INDr[m][:],
                                 Br[m][:, 480 * t:480 * t + ncol],
                                 start=(m == 0), stop=(m == 2))
            # permuted copy: psum (h', c, w') -> osb (c, h', w')
            dst = AP(osb.tensor, osb[:].offset + (5 * t) * 32,
                     [[3072, 64], [32, nrow], [1024, 3], [1, 32]])
            src_ap = AP(mm.tensor, mm[:].offset, [[480, 64], [96, nrow],
                                                  [32, 3], [1, 32]])
            if t % 2 == 0:
                nc.scalar.copy(dst, src_ap)
            else:
                nc.vector.tensor_copy(dst, src_ap)
        for c in range(C):
            for hh in range(2):
                for hq in range(2):
                    for wq in range(2):
                        q = 2 * hq + wq
                        dst = AP(o_d, c * 4096 + hh * 1024 + hq * 2048 + wq * 32,
                                 [[O_ROW, K], [PATCH, 16], [1, BLK]])
                        src_ap = AP(osb.tensor,
                                    osb[:].offset + (16 * q) * 3072
                                    + c * 1024 + hh * 512,
                                    [[3072, K], [BLK, 16], [1, BLK]])
                        eng = (c * 4 + q) % 3
                        (dma if eng == 0 else
                         dma_act if eng == 1 else dma_gp)(dst, src_ap)

    nc.compile()
    return nc


def _get_nc():
    if "nc" not in _CACHE:
        _CACHE["nc"] = _build_nc()
    return _CACHE["nc"]


def _run(x_high, scores_2d, noise, trace=False):
    from concourse import bass_utils
    nc = _get_nc()
    x_high = np.ascontiguousarray(x_high, dtype=np.float32)
    scores_2d = np.ascontiguousarray(scores_2d, dtype=np.float32)
    noise = np.ascontiguousarray(noise, dtype=np.float32)
    in_maps = [
        {"x": x_high[i], "sc": scores_2d[i], "nz": noise[i]}
        for i in range(NB)
    ]
    res = bass_utils.run_bass_kernel_spmd(
        nc, in_maps, core_ids=list(range(NB)), trace=trace)
    out = np.concatenate(
        [res.results[i]["o"].reshape(K, C, PATCH, PATCH) for i in range(NB)],
        axis=0)
    return out, res


def kernel(x_high, scores_2d, noise):
    out, _ = _run(x_high, scores_2d, noise, trace=False)
    return out



# revision 12
# speedup vs baseline: 1.1401x; 1.1401x over previous
"""DPS perturbed-top-k patch-extraction kernel for Trainium2 (Bass/Tile), v2.

Contract: kernel(**inputs) takes the FULL inputs
    x_high  (8, 3, 512, 512) f32
    scores_2d (8, 16, 16) f32
    noise   (8, 500, 256) f32
and returns the FULL output (128, 3, 64, 64) f32.

Sharding: pure data-parallel over batch b across the 8 NeuronCores (one
image per core). No collectives.

v2 redesign vs v1 (105us): the v1 timeline was a serial DMA chain
(zero-fill 4MB DRAM staging -> interior copy -> 18 B-load DMAs), with
57.9us of DMA-dispatch on the Sync engine alone. v2:
  * block-major DRAM staging xp[m] = [b][a2][h',c,w'] with B partition
    map p = 6b + a2, so each B tile loads with ONE 2-dim DMA
    (108 contiguous 12KB rows) instead of 18 3-dim DMAs.
  * zero-fill only the true pad regions (~680KB, disjoint from data ->
    fully concurrent with the data staging writes; no WAW serialization).
  * all tensors on the x path are f32r so no cast copies are needed
    before the 1-cyc/row PE matmul.
  * indicator: G_k(d') = #{n: cnt[n,d'] > k} computed directly with
    is_ge/Sign accumulation ops (split across DVE/ACT/GPSIMD), and
    ind_k(d') = (G_k(d') - G_k(d'-1)) * 1/500.  The per-form constant
    offsets (Sign form: 2G-512 scaled 1/1000) cancel in the d'-diff.
  * the (a2,b)->p permutation is absorbed for free into the 12 IND
    back-transposes via a strided free-axis access pattern.
  * output written as (64, 3072) quadrant-major rows (one 64x12KB DMA)
    and reassembled to (16,3,64,64) on the host (part of the allowed
    gather/unshard step).
"""
import numpy as np
from contextlib import ExitStack

# ---- problem constants (hardcoded per spec) ----
NB = 8           # batch / cores
C = 3
H = W = 512
HW = H * W
GS = 16          # score grid 16x16
GE = 18          # embedded grid stride
D2 = 256         # compact d
D3 = GE * GE     # 324
K = 16
N = 500
NCH = 4          # noise chunks
NP = 125         # rows per chunk
CM = 108         # partitions per block-chunk (18b x 6a2)
PATCH = 64
BLK = 32
SIG = 0.05
INV_N = 1.0 / 500.0
NEG = -1.0e30
F = C * BLK * BLK        # 3072 floats per block partition
SLAB = 6 * F             # 18432: per-b slab in xp[m]
XPM = 18 * SLAB          # 331776 floats per xp[m]
O_ROW = C * PATCH * PATCH  # 12288 floats per output patch

# G-phase k-assignment (contiguous ranges; ACT uses Sign form)
KV = list(range(0, 11))      # DVE: is_ge, scale 1/500
KG = []                      # (Pool engine lacks the accum-reduce ISA)
KA = list(range(11, 16))     # ACT: Sign, scale 1/1000, offset -0.512

_CACHE = {}


def _build_nc():
    import concourse.bacc as bacc
    import concourse.bass as bass
    import concourse.mybir as mybir
    import concourse.tile as tile

    F32 = mybir.dt.float32
    F32R = mybir.dt.float32r
    BF16 = mybir.dt.bfloat16
    I32 = mybir.dt.int32
    ALU = mybir.AluOpType
    ACTF = mybir.ActivationFunctionType
    AP = bass.AP

    nc = bacc.Bacc("TRN2", target_bir_lowering=False, debug=False)
    x_d = nc.dram_tensor("x", (C, H, W), F32R, kind="ExternalInput")
    sc_d = nc.dram_tensor("sc", (GS, GS), F32, kind="ExternalInput")
    nz_d = nc.dram_tensor("nz", (N, D2), F32, kind="ExternalInput")
    o_d = nc.dram_tensor("o", (64, F), F32, kind="ExternalOutput")
    xp = [nc.dram_tensor(f"xp{m}", (XPM,), F32R, kind="Internal")
          for m in range(3)]

    # per-m valid image-row ranges (r = 32a + h' - 16, a = 6m + a2)
    R0 = [0, 176, 368]
    NR = [176, 192, 144]
    # dst free offset of the first valid row inside an a2-slab
    OFF0 = [1536, 0, 0]   # m0 starts at a2=0,h'=16

    with tile.TileContext(nc) as tc, ExitStack() as ctx:
        sb = ctx.enter_context(tc.tile_pool(name="sb", bufs=1))
        ps_rep = ctx.enter_context(tc.tile_pool(name="ps_rep", bufs=1, space="PSUM"))
        ps_cnt = ctx.enter_context(tc.tile_pool(name="ps_cnt", bufs=1, space="PSUM"))
        ps_out = ctx.enter_context(tc.tile_pool(name="ps_out", bufs=3, space="PSUM"))

        def ap_of(t, off_elems, dims):
            return AP(t.tensor, t[:].offset + off_elems, dims)

        dma_s = nc.sync.dma_start
        dma_a = nc.scalar.dma_start
        dma_g = nc.gpsimd.dma_start

        # ---------------- early tiny tiles ----------------
        z32 = sb.tile([128, 512], F32)
        nc.vector.memset(z32[:], 0.0)
        zr = sb.tile([128, 512], F32R)
        nc.vector.tensor_copy(zr[:], z32[:])

        iota_t = sb.tile([128, 128], I32)
        nc.gpsimd.iota(iota_t[:], pattern=[[-1, 128]], base=0,
                       channel_multiplier=1)
        ident = sb.tile([128, 128], BF16)
        nc.vector.tensor_scalar(ident[:], iota_t[:], 0, None, op0=ALU.is_equal)
        diag05 = sb.tile([128, 128], F32)
        nc.vector.tensor_scalar(diag05[:], iota_t[:], 0, SIG,
                                op0=ALU.is_equal, op1=ALU.mult)
        ident_f32 = sb.tile([128, 128], F32)
        nc.vector.tensor_scalar(ident_f32[:], iota_t[:], 0, None,
                                op0=ALU.is_equal)
        # bias table for ACT Sign form: col k = -(k+0.5)
        iota_r = sb.tile([128, 17], I32)
        nc.gpsimd.iota(iota_r[:], pattern=[[-1, 17]], base=0,
                       channel_multiplier=0)
        # iota_r[p, j] = -j, so bias col j = -j - 0.5 = -(j + 0.5)
        bias_f = sb.tile([128, 17], F32)
        nc.vector.tensor_scalar(bias_f[:], iota_r[:], 1.0, -0.5,
                                op0=ALU.mult, op1=ALU.add)

        # A-tiles memset upfront (gpsimd, before its DMA dispatch work)
        A = [sb.tile([128, D3], F32, tag=f"A{t}", name=f"A{t}") for t in range(NCH)]
        for t in range(NCH):
            nc.gpsimd.memset(A[t][:], 0.0)

        # ---------------- scores pipeline (act loads, DVE computes) -----
        s256 = sb.tile([1, D2], F32)
        dma_a(s256[:], sc_d[:].rearrange("a b -> (a b)").unsqueeze(0))
        smax = sb.tile([1, 1], F32)
        smin = sb.tile([1, 1], F32)
        nc.vector.tensor_reduce(smax[:], s256[:], axis=mybir.AxisListType.X,
                                op=ALU.max)
        nc.vector.tensor_reduce(smin[:], s256[:], axis=mybir.AxisListType.X,
                                op=ALU.min)
        Dt = sb.tile([1, 1], F32)
        nc.vector.tensor_scalar(Dt[:], smax[:], smin[:], 1e-5,
                                op0=ALU.subtract, op1=ALU.add)
        rD = sb.tile([1, 1], F32)
        nc.vector.reciprocal(rD[:], Dt[:])
        s_row = sb.tile([1, D2], F32)
        nc.vector.tensor_scalar(s_row[:], s256[:], smin[:], rD[:],
                                op0=ALU.subtract, op1=ALU.mult)
        ones = sb.tile([1, 128], F32)
        nc.vector.memset(ones[:], 1.0)

        # ---------------- xp zero-fills (sync queue, m-ordered) ---------
        def zfill(eng, m, off, dims):
            # first dim reads real zr partitions; middle dims broadcast
            src_dims = ([[512, dims[0][1]]]
                        + [[0, d[1]] for d in dims[1:-1]]
                        + [[1, dims[-1][1]]])
            eng(AP(xp[m], off, dims),
                AP(zr.tensor, zr[:].offset, src_dims))

        def zero_m(m):
            # Z1: b=17 slab
            zfill(dma_s, m, 17 * SLAB, [[512, 36], [1, 512]])
            if m == 0:
                # Z3: a2=0 h'<16 row-pad, b 1..15
                zfill(dma_s, 0, SLAB + 0, [[SLAB, 15], [512, 3], [1, 512]])
            if m == 2:
                # Z2: a2=5 (a=17) slab for b 0..16
                zfill(dma_s, 2, 5 * F, [[SLAB, 17], [512, 6], [1, 512]])
                # Z4: a2=4 h'>=16 row-pad, b 1..15
                zfill(dma_s, 2, SLAB + 4 * F + 1536,
                      [[SLAB, 15], [512, 3], [1, 512]])
            na2 = 6 if m < 2 else 5
            # Z5: b=0 w'<16 col-pad (all a2,h',c in data range)
            zfill(dma_s, m, 0, [[F, na2], [32, 96], [1, 16]])
            # Z6: b=16 w'>=16 col-pad
            zfill(dma_s, m, 16 * SLAB + 16, [[F, na2], [32, 96], [1, 16]])

        # corners (act queue; tiny)
        def corners_m(m):
            if m == 0:
                zfill(dma_a, 0, 16, [[32, 48], [1, 16]])
                zfill(dma_a, 0, 16 * SLAB + 0, [[32, 48], [1, 16]])
            if m == 2:
                zfill(dma_a, 2, 4 * F + 1536 + 16, [[32, 48], [1, 16]])
                zfill(dma_a, 2, 16 * SLAB + 4 * F + 1536, [[32, 48], [1, 16]])

        # ---------------- xp data staging (gpsimd queue) ----------------
        def stage_m(m):
            r0, nr, off0 = R0[m], NR[m], OFF0[m]
            # left edge: cols 0..15 -> b=0, w' 16..31
            dma_g(AP(xp[m], off0 + 16, [[96, nr], [32, 3], [1, 16]]),
                  AP(x_d, r0 * W, [[W, nr], [HW, 3], [1, 16]]))
            # right edge: cols 496..511 -> b=16, w' 0..15
            dma_g(AP(xp[m], 16 * SLAB + off0, [[96, nr], [32, 3], [1, 16]]),
                  AP(x_d, r0 * W + 496, [[W, nr], [HW, 3], [1, 16]]))
            # interior: cols 16..495 -> b 1..15 (src (b,w') merges to 480)
            for c in range(C):
                dma_g(AP(xp[m], SLAB + off0 + 32 * c,
                         [[96, nr], [SLAB, 15], [1, 32]]),
                      AP(x_d, c * HW + r0 * W + 16,
                         [[W, nr], [32, 15], [1, 32]]))

        # B tiles + the one-DMA-per-tile loads (sync queue, after zeros)
        B = [sb.tile([CM, F], F32R, tag=f"B{m}", name=f"B{m}") for m in range(3)]

        def hop2_m(m):
            dma_s(ap_of(B[m], 0, [[F, CM], [1, F]]),
                  AP(xp[m], 0, [[F, CM], [1, F]]))

        # interleave: m0 zeros -> hop2 deferred until after stage_m(0)
        zero_m(0)
        corners_m(0)
        corners_m(2)
        stage_m(0)
        hop2_m(0)
        zero_m(1)
        stage_m(1)
        hop2_m(1)
        zero_m(2)
        stage_m(2)
        hop2_m(2)

        # ---------------- noise top-k -> cnt (per chunk) ----------------
        cnt = [sb.tile([128, D3], BF16, tag=f"cnt{t}", name=f"cnt{t}")
               for t in range(NCH)]
        for t in range(NCH):
            nz_t = sb.tile([128, D2], F32, tag=f"nz{t}", name=f"nzt{t}")
            dma_a(nz_t[0:NP, :], nz_d[NP * t:NP * (t + 1), :])
            pert_ps = ps_rep.tile([128, D2], F32, tag="pert_ps",
                                  name=f"pert_ps{t}", bufs=2)
            nc.tensor.matmul(pert_ps[0:NP, :], ones[:, 0:NP], s_row[:],
                             start=True, stop=False)
            nc.tensor.matmul(pert_ps[0:NP, :], diag05[0:NP, 0:NP],
                             nz_t[0:NP, :], start=False, stop=True)
            pert = sb.tile([128, D2], F32, tag=f"pert{t}", name=f"pert{t}")
            if t % 2 == 0:
                nc.scalar.copy(pert[0:NP, :], pert_ps[0:NP, :])
            else:
                nc.vector.tensor_copy(pert[0:NP, :], pert_ps[0:NP, :])
            top8 = sb.tile([128, 8], F32, tag=f"top8{t}", name=f"top8_{t}")
            nc.vector.max(top8[0:NP, :], pert[0:NP, :])
            pert2 = sb.tile([128, D2], F32, tag=f"pert2{t}", name=f"pert2_{t}")
            nc.vector.match_replace(pert2[0:NP, :], top8[0:NP, :],
                                    pert[0:NP, :], NEG)
            top8b = sb.tile([128, 8], F32, tag=f"top8b{t}", name=f"top8b_{t}")
            nc.vector.max(top8b[0:NP, :], pert2[0:NP, :])
            # A: embedded 18-stride 0/1 selection, then prefix-scan -> cnt
            a_view = ap_of(A[t], 0, [[D3, NP], [GE, GS], [1, GS]])
            p_view = ap_of(pert, 0, [[D2, NP], [GS, GS], [1, GS]])
            nc.vector.tensor_scalar(a_view, p_view, top8b[0:NP, 7:8], None,
                                    op0=ALU.is_ge)
            nc.vector.memset(cnt[t][:], 0.0)
            nc.vector.tensor_tensor_scan(cnt[t][0:NP, :], A[t][0:NP, :],
                                         A[t][0:NP, :], initial=0.0,
                                         op0=ALU.add, op1=ALU.bypass)

        # transposes into PSUM (d' on partitions, natural order), then
        # copy to SBUF so all three engines (incl. gpsimd) can read
        cntP = [ps_cnt.tile([CM, 512], BF16, tag=f"cntT{m}", name=f"cntTp{m}")
                for m in range(3)]
        for t in range(NCH):
            for m in range(3):
                nc.tensor.transpose(
                    cntP[m][:, 128 * t:128 * (t + 1)],
                    cnt[t][:, CM * m:CM * (m + 1)], ident[:])
        cntT = [sb.tile([CM, 512], BF16, tag=f"cntS{m}", name=f"cntS{m}")
                for m in range(3)]
        for m in range(3):
            if m % 2 == 0:
                nc.vector.tensor_copy(cntT[m][:], cntP[m][:])
            else:
                nc.scalar.copy(cntT[m][:], cntP[m][:])

        # ---------------- G accumulation (split DVE/GP/ACT) -------------
        G = [sb.tile([CM, K], F32, tag=f"G{m}", name=f"G{m}") for m in range(3)]
        scr_v = sb.tile([CM, 512], BF16, tag="scr_v", name="scr_v")
        scr_g = sb.tile([CM, 512], BF16, tag="scr_g", name="scr_g")
        scr_a = sb.tile([CM, 512], BF16, tag="scr_a", name="scr_a")
        for m in range(3):
            for k in KV:
                nc.vector.tensor_scalar(scr_v[:], cntT[m][:], float(k) + 0.5,
                                        None, op0=ALU.is_ge, op1=ALU.add,
                                        accum_out=G[m][:, k:k + 1])
            for k in KA:
                nc.scalar.activation(scr_a[:], cntT[m][:], ACTF.Sign,
                                     bias=bias_f[0:CM, k:k + 1], scale=1.0,
                                     accum_out=G[m][:, k:k + 1])

        # ---------------- gct: [16k, 1+324] scaled G forms --------------
        # per-k scale: 1/500 (is_ge rows), 1/1000 (Sign rows)
        # sign_row[k] = [k >= 12] (ACT rows); scvec = INV_N - 0.5*INV_N*sign_row
        ikf = sb.tile([16, 1], F32)
        nc.vector.tensor_copy(ikf[:], iota_t[0:16, 0:1])
        sgn = sb.tile([16, 1], F32)
        nc.vector.tensor_scalar(sgn[:], ikf[:], float(KA[0]) - 0.5, None,
                                op0=ALU.is_ge)
        scvec = sb.tile([16, 1], F32)
        nc.vector.tensor_scalar(scvec[:], sgn[:], -0.5 * INV_N, INV_N,
                                op0=ALU.mult, op1=ALU.add)
        gct_sb = sb.tile([16, 1 + D3], F32)
        # col0 = scaled G-form at d'=-1: 0 for is_ge rows, -512/1000 for Sign
        nc.vector.tensor_scalar(gct_sb[:, 0:1], sgn[:], -512.0 * 0.5 * INV_N,
                                None, op0=ALU.mult)
        for m in range(3):
            gct_ps = ps_rep.tile([16, CM], F32, tag="pert_ps",
                                 name=f"gct{m}", bufs=2)
            nc.tensor.transpose(gct_ps[:], G[m][:], ident_f32[0:CM, 0:CM])
            nc.vector.tensor_scalar(gct_sb[:, 1 + CM * m:1 + CM * (m + 1)],
                                    gct_ps[:], scvec[:], None, op0=ALU.mult)

        # indT_pad: cols 0..18 zero (quadrant shifts), col 19+d' = ind(d')
        indT_pad = sb.tile([16, 19 + D3], F32)
        nc.vector.memset(indT_pad[:, 0:19], 0.0)
        nc.vector.tensor_tensor(indT_pad[:, 19:19 + D3], gct_sb[:, 1:1 + D3],
                                gct_sb[:, 0:D3], op=ALU.subtract)

        # back-transposes with the (a2,b)->p=6b+a2 permutation in the
        # free-axis AP: element #p reads position base + b + 18*a2
        INDr = [sb.tile([CM, 64], F32R, tag=f"INDr{m}", name=f"INDr{m}")
                for m in range(3)]
        engs = [nc.vector.tensor_copy, nc.gpsimd.tensor_copy,
                lambda d, s_: nc.scalar.copy(d, s_)]
        for m in range(3):
            ind_ps = ps_cnt.tile([CM, 64], F32, tag=f"cntT{m}",
                                 name=f"indps{m}")
            for hq in range(2):
                for wq in range(2):
                    q = 2 * hq + wq
                    s = GE * hq + wq
                    # permuted copy: element #p of tmp = ind(d'(p) - s),
                    # p = 6b + a2  ->  free AP [[1,18b],[18,6a2]]
                    tmp = sb.tile([16, CM], F32, tag=f"iperm{q % 2}",
                                  name=f"iperm{m}_{q}")
                    src = AP(indT_pad.tensor,
                             indT_pad[:].offset + 19 + CM * m - s,
                             [[19 + D3, 16], [1, 18], [18, 6]])
                    engs[(2 * m + q) % 3](tmp[:], src)
                    nc.tensor.transpose(ind_ps[:, 16 * q:16 * (q + 1)],
                                        tmp[:], ident_f32[0:16, 0:16])
            if m % 2 == 0:
                nc.vector.tensor_copy(INDr[m][:], ind_ps[:])
            else:
                nc.scalar.copy(INDr[m][:], ind_ps[:])

        # ---------------- main matmul + output ----------------
        osb = sb.tile([64, F], F32)
        for t in range(7):
            ncol = 480 if t < 6 else 192
            mm = ps_out.tile([64, 480], F32, tag="mm", name=f"mm{t}")
            for m in range(3):
                nc.tensor.matmul(mm[:, 0:ncol], INDr[m][:],
                                 B[m][:, 480 * t:480 * t + ncol],
                                 start=(m == 0), stop=(m == 2))
            dst = osb[:, 480 * t:480 * t + ncol]
            src_ap = mm[:, 0:ncol]
            if t % 2 == 0:
                nc.scalar.copy(dst, src_ap)
            else:
                nc.vector.tensor_copy(dst, src_ap)
            if t == 3:
                dma_s(AP(o_d, 0, [[F, 64], [1, 1920]]),
                      ap_of(osb, 0, [[F, 64], [1, 1920]]))
        dma_s(AP(o_d, 1920, [[F, 64], [1, F - 1920]]),
              ap_of(osb, 1920, [[F, 64], [1, F - 1920]]))

    nc.compile()
    return nc


def _get_nc():
    if "nc" not in _CACHE:
        _CACHE["nc"] = _build_nc()
    return _CACHE["nc"]


def _unscramble(o2):
    # o2 (64, 3072) rows = (hq, wq, k), cols = (h', c, w')
    return (o2.reshape(2, 2, K, 32, C, 32)
              .transpose(2, 4, 0, 3, 1, 5)
              .reshape(K, C, PATCH, PATCH))


def _run(x_high, scores_2d, noise, trace=False):
    from concourse import bass_utils
    nc = _get_nc()
    x_high = np.ascontiguousarray(x_high, dtype=np.float32)
    scores_2d = np.ascontiguousarray(scores_2d, dtype=np.float32)
    noise = np.ascontiguousarray(noise, dtype=np.float32)
    in_maps = [
        {"x": x_high[i], "sc": scores_2d[i], "nz": noise[i]}
        for i in range(NB)
    ]
    res = bass_utils.run_bass_kernel_spmd(
        nc, in_maps, core_ids=list(range(NB)), trace=trace)
    out = np.concatenate(
        [_unscramble(np.asarray(res.results[i]["o"]))[None] for i in range(NB)],
        axis=0).reshape(NB * K, C, PATCH, PATCH)
    return out, res


def kernel(x_high, scores_2d, noise):
    out, _ = _run(x_high, scores_2d, noise, trace=False)
    return out
